# revision 7
# baseline (speedup 1.0000x reference)
"""Trainium2 Bass kernel for nn_InfiniAssociativeMemoryCell.

Contract: kernel(**inputs) takes the FULL unsharded inputs (numpy) keyed as in
setup_inputs(), returns the FULL [4, 4224, 1024] float32 output.

Sharding (hardcoded): 8 cores, core c -> batch b=c//2, token-half h=c%2.
Each core handles 256 "bulk" tokens per segment (h=0: rows 0..255 of the
segment, h=1: rows 256..511) for its batch, plus a fully redundant copy of the
recurrent memory update (the 16 memory tokens per segment) for its batch.
No cross-core communication. Memory-token output rows are taken from h=1 cores.

Math (validated vs reference in fp64/np):
  per segment s (state W_mem [128,1024], z [128]):
    bulk:  q2 = dpfp(hid @ W_mq); out = mo + tanh((q2 @ W_mem) / (q2 @ z + eps))
    rec:   assoc = (q2m @ W_mem) / (q2m @ z + eps); mem = assoc + mo_mem
           k2 = dpfp(mem @ W_mk); v = mem @ W_mv; gate = sigmoid(mem @ W_mb + b_mb)
           prev = (k2 @ W_mem) / (k2 @ z + eps)
           W_mem += k2.T @ (gate * (v - prev)); z += sum_t k2
All matmuls run in float32r (fp32 storage, reduced-precision PE compute).
"""
import numpy as np
from contextlib import ExitStack

import concourse.bacc as bacc
import concourse.tile as tile
import concourse.mybir as mybir
from concourse import bass_utils
from concourse.masks import make_identity

EPS = 1e-5
NMEM = 16
SEG = 512
L = SEG + NMEM          # 528
NSEG = 8
B, S, D, DM = 4, 4224, 1024, 64
DK = 2 * DM             # 128
NC = 8                  # cores
BULK = 256              # bulk tokens per (core, segment)
NBROWS = BULK * NSEG    # 2048 bulk rows per core
NMROWS = NMEM * NSEG    # 128 mem rows per core
NCHUNK = D // 128       # 8 contraction chunks

f32 = mybir.dt.float32
f32r = mybir.dt.float32r
AF = mybir.ActivationFunctionType


def build_nc():
    """Build the SPMD Bass program for one core."""
    nc = bacc.Bacc("TRN2", target_bir_lowering=False, debug=False)

    hid_bulk = nc.dram_tensor("hid_bulk", (NBROWS, D), f32, kind="ExternalInput")
    mo_bulk = nc.dram_tensor("mo_bulk", (NBROWS, D), f32, kind="ExternalInput")
    hid_mem = nc.dram_tensor("hid_mem", (NMROWS, D), f32, kind="ExternalInput")
    mo_mem = nc.dram_tensor("mo_mem", (NMROWS, D), f32, kind="ExternalInput")
    w_mq = nc.dram_tensor("W_mq", (D, DM), f32, kind="ExternalInput")
    w_mk = nc.dram_tensor("W_mk", (D, DM), f32, kind="ExternalInput")
    w_mv = nc.dram_tensor("W_mv", (D, D), f32, kind="ExternalInput")
    w_mb = nc.dram_tensor("W_mb", (D, D), f32, kind="ExternalInput")
    b_mb = nc.dram_tensor("b_mb", (D,), f32, kind="ExternalInput")
    out = nc.dram_tensor("out", (NBROWS + NMROWS, D), f32, kind="ExternalOutput")

    with tile.TileContext(nc) as tc, ExitStack() as ctx:
        # ---------------- pools ----------------
        wts = ctx.enter_context(tc.tile_pool(name="wts", bufs=1))
        state = ctx.enter_context(tc.tile_pool(name="state", bufs=2))
        memstat = ctx.enter_context(tc.tile_pool(name="memstat", bufs=1))
        p_in = ctx.enter_context(tc.tile_pool(name="p_in", bufs=3))
        p_mo = ctx.enter_context(tc.tile_pool(name="p_mo", bufs=3))
        p_hT = ctx.enter_context(tc.tile_pool(name="p_hT", bufs=2))
        p_small = ctx.enter_context(tc.tile_pool(name="p_small", bufs=2))
        p_ta = ctx.enter_context(tc.tile_pool(name="p_ta", bufs=2))
        p_out = ctx.enter_context(tc.tile_pool(name="p_out", bufs=2))
        p_rec = ctx.enter_context(tc.tile_pool(name="p_rec", bufs=1))
        p_rec2 = ctx.enter_context(tc.tile_pool(name="p_rec2", bufs=1))

        ps_big = ctx.enter_context(tc.tile_pool(name="ps_big", bufs=3, space="PSUM"))
        ps_small = ctx.enter_context(tc.tile_pool(name="ps_small", bufs=2, space="PSUM"))

        # ---------------- constants & weights ----------------
        ident_f = wts.tile([128, 128], f32)
        make_identity(nc, ident_f[:])
        ident = wts.tile([128, 128], f32r)
        nc.vector.tensor_copy(ident[:], ident_f[:])
        ones16_f = wts.tile([1, NMEM], f32)
        nc.vector.memset(ones16_f[:], 1.0)
        ones16 = wts.tile([1, NMEM], f32r)
        nc.vector.tensor_copy(ones16[:], ones16_f[:])

        wmq_sb = wts.tile([128, NCHUNK, DM], f32r)
        nc.gpsimd.dma_start(wmq_sb[:], w_mq.ap().rearrange("(c p) m -> p c m", p=128))
        wmk_sb = wts.tile([128, NCHUNK, DM], f32r)
        nc.gpsimd.dma_start(wmk_sb[:], w_mk.ap().rearrange("(c p) m -> p c m", p=128))
        wmv_sb = wts.tile([128, NCHUNK, D], f32r)
        nc.gpsimd.dma_start(wmv_sb[:], w_mv.ap().rearrange("(c p) m -> p c m", p=128))
        wmb_sb = wts.tile([128, NCHUNK, D], f32r)
        nc.gpsimd.dma_start(wmb_sb[:], w_mb.ap().rearrange("(c p) m -> p c m", p=128))
        bmb_sb = wts.tile([1, D], f32r)
        nc.gpsimd.dma_start(bmb_sb[:], b_mb.ap().rearrange("(one n) -> one n", one=1))

        # ---------------- state ----------------
        zero_f = p_ta.tile([128, D], f32, tag="ta")
        nc.vector.memset(zero_f[:], 0.0)
        wmem = state.tile([128, D], f32r, tag="wmem")
        nc.vector.tensor_copy(wmem[:], zero_f[:])
        zvec = state.tile([128, 2], f32r, tag="zvec")
        nc.vector.tensor_copy(zvec[:], zero_f[:, 0:2])

        def transpose_128x1024_to_chunks(src_sb, dst_sb, dtype):
            """src [128, 1024] -> dst [128, 8, 128] (dst[p, c, t] = src[t, 128c+p])."""
            t_ps = ps_big.tile([128, 1024], dtype, tag="big")
            for c in range(NCHUNK):
                nc.tensor.transpose(
                    t_ps[:, 128 * c:128 * (c + 1)],
                    src_sb[:, 128 * c:128 * (c + 1)], ident[:])
            nc.scalar.copy(dst_sb[:].rearrange("p c t -> p (c t)"), t_ps[:])

        def q2_from_hidT(hT_sb, name_pool_small):
            """hT [128, 8, 128](f32r) -> (q2_sb [128,128] f32r, q2T_sb [128,128] f32r).

            q2 rows = tokens. dpfp along free dim: x2 = [relu(q), relu(-q)],
            q2 = x2 * roll(x2, 1)."""
            q_ps = ps_small.tile([128, DM], f32, tag="small")
            for c in range(NCHUNK):
                nc.tensor.matmul(q_ps[:], hT_sb[:, c, :], wmq_sb[:, c, :],
                                 start=(c == 0), stop=(c == NCHUNK - 1))
            x2 = name_pool_small.tile([128, DK], f32, tag="x2")
            nc.scalar.activation(x2[:, 0:DM], q_ps[:], AF.Relu)
            nc.scalar.activation(x2[:, DM:DK], q_ps[:], AF.Relu, scale=-1.0)
            q2 = name_pool_small.tile([128, DK], f32r, tag="q2")
            nc.vector.tensor_mul(q2[:, 1:DK], x2[:, 1:DK], x2[:, 0:DK - 1])
            nc.vector.tensor_mul(q2[:, 0:1], x2[:, 0:1], x2[:, DK - 1:DK])
            q2T_ps = ps_small.tile([128, 128], f32r, tag="small")
            nc.tensor.transpose(q2T_ps[:], q2[:], ident[:])
            q2T = name_pool_small.tile([128, 128], f32r, tag="q2T")
            nc.vector.tensor_copy(q2T[:], q2T_ps[:])
            return q2, q2T

        # ---------------- upfront: q2T for all mem tokens ----------------
        hidm_sb = memstat.tile([128, D], f32r)
        nc.gpsimd.dma_start(hidm_sb[:], hid_mem[:, :])
        mom_sb = memstat.tile([16, NSEG, D], f32)
        nc.sync.dma_start(mom_sb[:], mo_mem.ap().rearrange("(s p) d -> p s d", p=16))
        hmT_sb = memstat.tile([128, NCHUNK, 128], f32r)
        transpose_128x1024_to_chunks(hidm_sb, hmT_sb, f32r)
        _, q2Tm = q2_from_hidT(hmT_sb, memstat)
        # q2Tm columns: token index = 16*s + i  (seg-major)

        # ---------------- main loop over segments ----------------
        for s in range(NSEG):
            # ======== recurrence for segment s (updates W_mem, z) ========
            q2Ts = q2Tm[:, NMEM * s:NMEM * (s + 1)]     # [128, 16] lhsT
            den_ps = ps_small.tile([16, 8], f32, tag="small")
            nc.tensor.matmul(den_ps[:, 0:2], q2Ts, zvec[:], start=True, stop=True)
            rden = p_rec.tile([16, 1], f32, tag="rden")
            nc.vector.tensor_scalar_add(rden[:], den_ps[:, 0:1], EPS)
            nc.vector.reciprocal(rden[:], rden[:])

            num_ps = ps_big.tile([16, D], f32, tag="big")
            for half in range(2):
                nc.tensor.matmul(num_ps[:, 512 * half:512 * (half + 1)],
                                 q2Ts, wmem[:, 512 * half:512 * (half + 1)],
                                 start=True, stop=True)
            assoc = p_rec.tile([16, D], f32, tag="assoc")
            nc.scalar.activation(assoc[:], num_ps[:], AF.Copy, scale=rden[:])
            # output rows for the mem tokens (h=1 cores' values are used)
            tanh_mem = p_rec2.tile([16, D], f32, tag="tanh_mem")
            nc.scalar.activation(tanh_mem[:], num_ps[:], AF.Tanh, scale=rden[:])
            outmem = p_rec2.tile([16, D], f32, tag="outmem")
            nc.vector.tensor_add(outmem[:], tanh_mem[:], mom_sb[:, s, :])
            nc.sync.dma_start(
                out[NBROWS + NMEM * s:NBROWS + NMEM * (s + 1), :], outmem[:])

            mem = p_rec.tile([16, D], f32r, tag="mem")
            nc.vector.tensor_add(mem[:], assoc[:], mom_sb[:, s, :])

            # memT [128, 8, 16]
            memT_ps = ps_small.tile([128, 128], f32r, tag="small")
            for c in range(NCHUNK):
                nc.tensor.transpose(memT_ps[:, 16 * c:16 * (c + 1)],
                                    mem[:, 128 * c:128 * (c + 1)],
                                    ident[0:16, 0:16])
            memT = p_rec.tile([128, NCHUNK, NMEM], f32r, tag="memT")
            nc.vector.tensor_copy(
                memT[:].rearrange("p c t -> p (c t)"), memT_ps[:])

            # k projection + dpfp
            kraw_ps = ps_small.tile([16, DM], f32, tag="small")
            for c in range(NCHUNK):
                nc.tensor.matmul(kraw_ps[:], memT[:, c, :], wmk_sb[:, c, :],
                                 start=(c == 0), stop=(c == NCHUNK - 1))
            xk = p_rec.tile([16, DK], f32, tag="xk")
            nc.scalar.activation(xk[:, 0:DM], kraw_ps[:], AF.Relu)
            nc.scalar.activation(xk[:, DM:DK], kraw_ps[:], AF.Relu, scale=-1.0)
            k2 = p_rec.tile([16, DK], f32r, tag="k2")
            nc.vector.tensor_mul(k2[:, 1:DK], xk[:, 1:DK], xk[:, 0:DK - 1])
            nc.vector.tensor_mul(k2[:, 0:1], xk[:, 0:1], xk[:, DK - 1:DK])
            k2T_ps = ps_small.tile([128, 16], f32r, tag="small")
            nc.tensor.transpose(k2T_ps[:], k2[:], ident[0:16, 0:16])
            k2T = p_rec.tile([128, NMEM], f32r, tag="k2T")
            nc.vector.tensor_copy(k2T[:], k2T_ps[:])

            # v / gate streams
            v_ps = ps_big.tile([16, D], f32, tag="big")
            for half in range(2):
                for c in range(NCHUNK):
                    nc.tensor.matmul(v_ps[:, 512 * half:512 * (half + 1)],
                                     memT[:, c, :],
                                     wmv_sb[:, c, 512 * half:512 * (half + 1)],
                                     start=(c == 0), stop=(c == NCHUNK - 1))
            g_ps = ps_big.tile([16, D], f32, tag="big")
            for half in range(2):
                for c in range(NCHUNK):
                    nc.tensor.matmul(g_ps[:, 512 * half:512 * (half + 1)],
                                     memT[:, c, :],
                                     wmb_sb[:, c, 512 * half:512 * (half + 1)],
                                     start=(c == 0), stop=False)
                nc.tensor.matmul(g_ps[:, 512 * half:512 * (half + 1)],
                                 ones16[:],
                                 bmb_sb[:, 512 * half:512 * (half + 1)],
                                 start=False, stop=True)
            gate = p_rec.tile([16, D], f32, tag="gate")
            nc.scalar.activation(gate[:], g_ps[:], AF.Sigmoid)

            # prev = (k2 @ W_mem) / (k2 @ z + eps)
            den2_ps = ps_small.tile([16, 8], f32, tag="small")
            nc.tensor.matmul(den2_ps[:, 0:2], k2T[:], zvec[:], start=True, stop=True)
            rden2 = p_rec.tile([16, 1], f32, tag="rden2")
            nc.vector.tensor_scalar_add(rden2[:], den2_ps[:, 0:1], EPS)
            nc.vector.reciprocal(rden2[:], rden2[:])
            num2_ps = ps_big.tile([16, D], f32, tag="big")
            for half in range(2):
                nc.tensor.matmul(num2_ps[:, 512 * half:512 * (half + 1)],
                                 k2T[:], wmem[:, 512 * half:512 * (half + 1)],
                                 start=True, stop=True)
            prev = p_rec.tile([16, D], f32, tag="prev")
            nc.scalar.activation(prev[:], num2_ps[:], AF.Copy, scale=rden2[:])

            vsub = p_rec.tile([16, D], f32, tag="vsub")
            nc.vector.tensor_sub(vsub[:], v_ps[:], prev[:])
            ninfo = p_rec.tile([16, D], f32r, tag="ninfo")
            nc.vector.tensor_mul(ninfo[:], gate[:], vsub[:])

            # W_mem update
            updW_ps = ps_big.tile([128, D], f32, tag="big")
            for half in range(2):
                nc.tensor.matmul(updW_ps[:, 512 * half:512 * (half + 1)],
                                 k2[:], ninfo[:, 512 * half:512 * (half + 1)],
                                 start=True, stop=True)
            wmem_new = state.tile([128, D], f32r, tag="wmem")
            nc.vector.tensor_add(wmem_new[:], wmem[:].bitcast(f32), updW_ps[:])
            zred = p_rec.tile([128, 1], f32, tag="zred")
            nc.vector.reduce_sum(zred[:], k2T[:].bitcast(f32),
                                 axis=mybir.AxisListType.X)
            zvec_new = state.tile([128, 2], f32r, tag="zvec")
            nc.vector.tensor_scalar_add(zvec_new[:], zvec[:].bitcast(f32), zred[:])

            # ======== bulk: 2 tiles of 128 tokens, read W_mem/z of seg s ========
            for t in range(2):
                r0 = BULK * s + 128 * t
                hid_t = p_in.tile([128, D], f32r, tag="hid")
                nc.gpsimd.dma_start(hid_t[:], hid_bulk[r0:r0 + 128, :])
                mo_t = p_mo.tile([128, D], f32, tag="mo")
                nc.sync.dma_start(mo_t[:], mo_bulk[r0:r0 + 128, :])

                hT_sb = p_hT.tile([128, NCHUNK, 128], f32r, tag="hT")
                transpose_128x1024_to_chunks(hid_t, hT_sb, f32r)
                q2, q2T = q2_from_hidT(hT_sb, p_small)

                den_b_ps = ps_small.tile([128, 8], f32, tag="small")
                nc.tensor.matmul(den_b_ps[:, 0:2], q2T[:], zvec[:],
                                 start=True, stop=True)
                rden_b = p_small.tile([128, 1], f32, tag="rden_b")
                nc.vector.tensor_scalar_add(rden_b[:], den_b_ps[:, 0:1], EPS)
                nc.vector.reciprocal(rden_b[:], rden_b[:])

                numb_ps = ps_big.tile([128, D], f32, tag="big")
                for half in range(2):
                    nc.tensor.matmul(numb_ps[:, 512 * half:512 * (half + 1)],
                                     q2T[:], wmem[:, 512 * half:512 * (half + 1)],
                                     start=True, stop=True)
                ta = p_ta.tile([128, D], f32, tag="ta")
                nc.scalar.activation(ta[:], numb_ps[:], AF.Tanh, scale=rden_b[:])
                out_t = p_out.tile([128, D], f32, tag="out")
                nc.vector.tensor_add(out_t[:], ta[:], mo_t[:])
                nc.sync.dma_start(out[r0:r0 + 128, :], out_t[:])

            # rotate state
            wmem = wmem_new
            zvec = zvec_new

    nc.compile()
    return nc


def shard_inputs(hidden_states, model_output, W_mq, W_mk, W_mv, W_mb, b_mb):
    hs = np.ascontiguousarray(hidden_states, dtype=np.float32)
    mo = np.ascontiguousarray(model_output, dtype=np.float32)
    bulk_idx = {}
    mem_idx = np.concatenate([np.arange(L * s + SEG, L * s + L) for s in range(NSEG)])
    for h in range(2):
        bulk_idx[h] = np.concatenate(
            [np.arange(L * s + BULK * h, L * s + BULK * (h + 1)) for s in range(NSEG)])
    in_maps = []
    for c in range(NC):
        b, h = c // 2, c % 2
        in_maps.append({
            "hid_bulk": np.ascontiguousarray(hs[b][bulk_idx[h]]),
            "mo_bulk": np.ascontiguousarray(mo[b][bulk_idx[h]]),
            "hid_mem": np.ascontiguousarray(hs[b][mem_idx]),
            "mo_mem": np.ascontiguousarray(mo[b][mem_idx]),
            "W_mq": np.ascontiguousarray(W_mq, dtype=np.float32),
            "W_mk": np.ascontiguousarray(W_mk, dtype=np.float32),
            "W_mv": np.ascontiguousarray(W_mv, dtype=np.float32),
            "W_mb": np.ascontiguousarray(W_mb, dtype=np.float32),
            "b_mb": np.ascontiguousarray(b_mb, dtype=np.float32),
        })
    return in_maps


def assemble(outs):
    full = np.zeros((B, S, D), np.float32)
    for c in range(NC):
        b, h = c // 2, c % 2
        o = outs[c]["out"]
        for s in range(NSEG):
            full[b, L * s + BULK * h: L * s + BULK * (h + 1)] = \
                o[BULK * s:BULK * (s + 1)]
            if h == 1:
                full[b, L * s + SEG: L * s + L] = \
                    o[NBROWS + NMEM * s: NBROWS + NMEM * (s + 1)]
    return full


_NC_CACHE = None


def get_nc():
    global _NC_CACHE
    if _NC_CACHE is None:
        _NC_CACHE = build_nc()
    return _NC_CACHE


def kernel(**inputs):
    nc = get_nc()
    in_maps = shard_inputs(**inputs)
    res = bass_utils.run_bass_kernel_spmd(nc, in_maps, core_ids=list(range(NC)))
    return assemble(res.results)
